# revision 34
# baseline (speedup 1.0000x reference)
# Deformable-conv (DCNv2-style, scrambled-reshape variant) Trainium2 Bass kernel.
# Data-parallel over batch: 8 samples -> 8 NeuronCores.
#
# Per-core pipeline (all layouts derived + validated against the reference in numpy):
#   1. offset conv (18ch, fp16) over padded x -> PE-transpose -> per-n2 selection
#      matmuls -> flat 2x2-patch index f00 + bilinear fracs; gathers can start as
#      soon as idxt[n2] lands. Modulation conv (9ch) over padded x^T + scale
#      table build run behind the first gathers.
#   2. 16 indirect-DMA gathers per (sp, n2) from a host-built patch table
#      (row f = [128 ch x 4 corners] of flat pixels [f, f+1, f+64, f+65], fp16,
#      corner-minor so the scale multiply runs at 2x DVE rate).
#   3. Combine: 2 half-tile muls by (modulation x bilinear) scales, 2 half-tile
#      corner-pair adds (all packed fp16, 2x DVE), one strided final add.
#   4. PE-transpose back to channel-major, Act drains PSUM into vc.
#   5. Main conv = 9 accumulated fp16 matmuls per output tile; Act PSUM copies
#      write through a transposed AP to undo the pi2' ordering.
import sys

import numpy as np

sys.path.insert(0, "/opt/trn_rl_repo")

import concourse.bass as bass
import concourse.bacc as bacc
import concourse.mybir as mybir
from concourse import tile
from concourse.bass_utils import run_bass_kernel_spmd

F32 = mybir.dt.float32
F16 = mybir.dt.float16
I32 = mybir.dt.int32

B, C, H, W = 8, 128, 64, 64
OUT = 256
PIX = H * W            # 4096
KCH = 32               # pixel-major chunks (4096 / 128)
TROWS = 4224           # patch table rows (4096 + pad for f+65 reads)

_CACHE = {}


def _build_host_constants():
    if "sel" in _CACHE:
        return _CACHE
    p2 = np.arange(128)
    k2 = np.arange(KCH)
    sel = np.zeros((9, 3, 128, 128), np.float16)   # [n2, r, p_src, p2]
    basey = np.zeros((9, 128, KCH), np.float32)
    basex = np.zeros((9, 128, KCH), np.float32)
    for n2 in range(9):
        a2, e2 = n2 // 3, n2 % 3
        i2 = p2 % 64
        r = (i2 + e2) % 3
        n = 3 * r + a2                       # source kernel point per partition
        J = (64 * e2 + i2) // 3              # source col j per partition
        c_src = 64 * (p2 // 64) + J          # source partition in pixel-major
        for rr in range(3):
            m = r == rr
            sel[n2, rr, c_src[m], p2[m]] = 1.0
        a = n // 3
        e = n % 3
        # y_u = i + a + o_y ; i = j2 = 2*k2 + p2//64
        basey[n2] = (2 * k2[None, :] + (p2 // 64)[:, None]) + a[:, None]
        basex[n2] = (J + e)[:, None] * np.ones((1, KCH), np.float32)
    _CACHE["sel"] = np.ascontiguousarray(
        sel.transpose(2, 0, 1, 3)).reshape(128, 9 * 3 * 128)
    _CACHE["basey"] = np.ascontiguousarray(
        basey.transpose(1, 0, 2)).reshape(128, 9 * KCH)
    _CACHE["basex"] = np.ascontiguousarray(
        basex.transpose(1, 0, 2)).reshape(128, 9 * KCH)
    _CACHE["ident16"] = np.eye(128, dtype=np.float16)
    return _CACHE


def _pad66(img):  # [C,64,64] -> [C, 66*66] zero-padded
    p = np.zeros((C, 66, 66), np.float16)
    p[:, 1:65, 1:65] = img
    return p.reshape(C, 66 * 66)


def _patch_table(img):  # [C,64,64] f32 -> [TROWS, 512] fp16, rows [ch, corner]
    flat = np.zeros((C, TROWS + 65), np.float16)
    flat[:, :PIX] = img.reshape(C, PIX).astype(np.float16)
    f = np.arange(TROWS)
    tab = np.stack(
        [flat[:, f], flat[:, f + 1], flat[:, f + 64], flat[:, f + 65]], axis=2
    )  # [C, TROWS, 4]
    return np.ascontiguousarray(tab.transpose(1, 0, 2)).reshape(TROWS, 512)


def _build_program():
    if "nc" in _CACHE:
        return _CACHE["nc"]
    nc = bacc.Bacc()
    d = {}
    d["xpad"] = nc.dram_tensor("xpad", [C, 66 * 66], F16, kind="ExternalInput")
    d["xtpad"] = nc.dram_tensor("xtpad", [C, 66 * 66], F16, kind="ExternalInput")
    d["ptab"] = nc.dram_tensor("ptab", [TROWS, 512], F16, kind="ExternalInput")
    d["wom"] = nc.dram_tensor("wom", [C, 9 * 18], F16, kind="ExternalInput")
    d["wmt"] = nc.dram_tensor("wmt", [C, 9 * 9], F16, kind="ExternalInput")
    d["ob"] = nc.dram_tensor("ob", [18, 1], F32, kind="ExternalInput")
    d["mb"] = nc.dram_tensor("mb", [9, 1], F32, kind="ExternalInput")
    d["sel"] = nc.dram_tensor("sel", [128, 9 * 3 * 128], F16, kind="ExternalInput")
    d["basey"] = nc.dram_tensor("basey", [128, 9 * KCH], F32, kind="ExternalInput")
    d["basex"] = nc.dram_tensor("basex", [128, 9 * KCH], F32, kind="ExternalInput")
    d["w2"] = nc.dram_tensor("w2", [C, 9 * 2 * 128], F16, kind="ExternalInput")
    d["id16"] = nc.dram_tensor("id16", [128, 128], F16, kind="ExternalInput")
    d["out"] = nc.dram_tensor("out", [OUT, PIX], F16, kind="ExternalOutput")

    AO = mybir.AluOpType

    with tile.TileContext(nc) as tc:
        with (
            tc.tile_pool(name="imgs", bufs=1) as imgs,
            tc.tile_pool(name="wts", bufs=1) as wts,
            tc.tile_pool(name="meta", bufs=1) as meta,
            tc.tile_pool(name="big", bufs=2) as big,
            tc.tile_pool(name="ps", bufs=2, space="PSUM") as psp,
            tc.tile_pool(name="pst", bufs=2, space="PSUM") as pst,
            tc.tile_pool(name="gbuf", bufs=2) as gbuf,
            tc.tile_pool(name="vbuf", bufs=2) as vbuf,
            tc.tile_pool(name="obuf", bufs=2) as obuf,
        ):
            # ---- loads: conv1/idx path first, conv2/scale + main-conv later
            xpad = imgs.tile([C, 66 * 66], F16)
            xtpad = imgs.tile([C, 66 * 66], F16)
            wom = wts.tile([C, 9, 18], F16)
            wmt = wts.tile([C, 9, 9], F16)
            ob = wts.tile([18, 1], F32)
            mb = wts.tile([9, 1], F32)
            id16 = wts.tile([128, 128], F16)
            selt = wts.tile([128, 9, 3, 128], F16)
            basey = wts.tile([128, 9, KCH], F32)
            basex = wts.tile([128, 9, KCH], F32)
            w2 = wts.tile([C, 9, 2, 128], F16)
            nc.sync.dma_start(id16[:], d["id16"][:])
            nc.sync.dma_start(xpad[:, 0:2244], d["xpad"][:, 0:2244])
            nc.sync.dma_start(wom[:], d["wom"][:])
            nc.sync.dma_start(ob[:], d["ob"][:])
            nc.sync.dma_start(xpad[:, 2244:], d["xpad"][:, 2244:])
            nc.sync.dma_start(selt[:], d["sel"][:])
            nc.sync.dma_start(basey[:], d["basey"][:])
            nc.sync.dma_start(basex[:], d["basex"][:])
            nc.sync.dma_start(wmt[:], d["wmt"][:])
            nc.sync.dma_start(mb[:], d["mb"][:])
            nc.sync.dma_start(xtpad[:], d["xtpad"][:])
            nc.sync.dma_start(w2[:], d["w2"][:])

            # ---- PE p-state warmup on the identity while xpad streams in
            for _ in range(24):
                wpt = pst.tile([128, 128], F16, tag="tv", name="wpt")
                nc.tensor.transpose(wpt[:], id16[:], id16[:])

            # ---- conv1 (offsets, 18ch over xpad), transposes interleaved
            ocm = big.tile([128, PIX], F16, tag="big")
            opm = meta.tile([128, KCH, 18], F16)   # pi = 128k+p

            def conv1_half(hk):
                for tl in range(4 * hk, 4 * hk + 4):
                    po = psp.tile([18, 512], F32, tag="mm", name="po")
                    for t in range(9):
                        dy, dx = t // 3, t % 3
                        off = dy * 66 + dx + tl * 8 * 66
                        rhs1 = bass.AP(
                            tensor=xpad[:].tensor, offset=xpad[:].offset + off,
                            ap=[list(xpad[:].ap[0]), [66, 8], [1, 64]],
                        )
                        nc.tensor.matmul(po[:], wom[:, t, :], rhs1,
                                         start=(t == 0), stop=(t == 8))
                    nc.scalar.activation(ocm[0:18, tl * 512:(tl + 1) * 512],
                                         po[:],
                                         mybir.ActivationFunctionType.Identity,
                                         bias=ob[:], scale=1.0)
                    for k in range(4 * tl, 4 * tl + 4):
                        pt = pst.tile([128, 128], F16, tag="tr", name="pt")
                        nc.tensor.transpose(pt[:], ocm[:, k * 128:(k + 1) * 128],
                                            id16[:])
                        nc.scalar.copy(opm[:, k, :], pt[:, 0:18])

            # ---- per-n2: selection matmuls -> positions -> idx + fracs.
            # Split by k-half: half 0 covers sp=0's chunks, so its gathers
            # start after only half the pipeline latency.
            idxt = meta.tile([128, 9, KCH], I32)
            sb4 = meta.tile([128, 9, KCH, 4], F32)
            KH = KCH // 2

            def meta_half(hk):
                for n2 in range(9):
                    oyx = pst.tile([128, KH, 2], F32, tag="oyx", name="oyx")
                    for r in range(3):
                        a2 = n2 // 3
                        ch = 3 * r + a2
                        rhs = bass.AP(
                            tensor=opm[:].tensor,
                            offset=opm[:].offset + ch + hk * KH * 18,
                            ap=[list(opm[:].ap[0]), [18, KH], [9, 2]],
                        )
                        nc.tensor.matmul(oyx[:], selt[:, n2, r, :], rhs,
                                         start=(r == 0), stop=(r == 2))
                    kr = slice(hk * KH, (hk + 1) * KH)
                    P = meta.tile([128, KH, 2], F32, tag="P", name="P")
                    nc.vector.tensor_add(P[:, :, 0], oyx[:, :, 0],
                                         basey[:, n2, kr])
                    nc.vector.tensor_add(P[:, :, 1], oyx[:, :, 1],
                                         basex[:, n2, kr])
                    nc.vector.tensor_scalar(P[:], P[:], 0.0, 63.0,
                                            AO.max, AO.min)
                    R0 = meta.tile([128, KH, 2], F32, tag="R0", name="R0")
                    nc.vector.tensor_scalar(R0[:], P[:], -0.5, 12582912.0,
                                            AO.add, AO.add)
                    nc.vector.tensor_scalar_add(R0[:], R0[:], -12582912.0)
                    Fh = meta.tile([128, KH, 2], F32, tag="Fh", name="Fh")
                    nc.vector.tensor_sub(Fh[:], P[:], R0[:])
                    nc.vector.scalar_tensor_tensor(
                        idxt[:, n2, kr], R0[:, :, 1], 64.0, R0[:, :, 0],
                        AO.mult, AO.add)
                    # bilinear-only corner weights (modulation folded later):
                    # c0=(1-F1)(1-F0) c1=(1-F1)F0 c2=F1(1-F0) c3=F1*F0
                    nc.vector.tensor_scalar(sb4[:, n2, kr, 0], Fh[:, :, 1],
                                            -1.0, 1.0, AO.mult, AO.add)
                    nc.vector.tensor_mul(sb4[:, n2, kr, 1], sb4[:, n2, kr, 0],
                                         Fh[:, :, 0])
                    nc.vector.tensor_sub(sb4[:, n2, kr, 0], sb4[:, n2, kr, 0],
                                         sb4[:, n2, kr, 1])
                    nc.vector.tensor_mul(sb4[:, n2, kr, 3], Fh[:, :, 1],
                                         Fh[:, :, 0])
                    nc.vector.tensor_sub(sb4[:, n2, kr, 2], Fh[:, :, 1],
                                         sb4[:, n2, kr, 3])

            # ---- conv2 (modulation, 9ch over xtpad) + sigmoid, per-half
            mcm = big.tile([128, PIX], F16, tag="big")
            mpm = meta.tile([128, KCH, 9], F32)    # pi2' = 128k+p

            def conv2_half(hk):
                for tl in range(4 * hk, 4 * hk + 4):
                    pm = psp.tile([9, 512], F32, tag="mm", name="pm")
                    for t in range(9):
                        dy, dx = t // 3, t % 3
                        off = dy * 66 + dx + tl * 8 * 66
                        rhs2 = bass.AP(
                            tensor=xtpad[:].tensor,
                            offset=xtpad[:].offset + off,
                            ap=[list(xtpad[:].ap[0]), [66, 8], [1, 64]],
                        )
                        nc.tensor.matmul(pm[:], wmt[:, t, :], rhs2,
                                         start=(t == 0), stop=(t == 8))
                    nc.scalar.activation(mcm[0:9, tl * 512:(tl + 1) * 512],
                                         pm[:],
                                         mybir.ActivationFunctionType.Sigmoid,
                                         bias=mb[:], scale=1.0)
                    for k in range(4 * tl, 4 * tl + 4):
                        pt2 = pst.tile([128, 128], F16, tag="tr", name="pt2")
                        nc.tensor.transpose(pt2[:],
                                            mcm[:, k * 128:(k + 1) * 128],
                                            id16[:])
                        nc.scalar.copy(mpm[:, k, :], pt2[:, 0:9])

            # ---- scales: fold modulation into bilinear weights (tiny
            # broadcast muls; conv2-dependent work kept off the combine path)
            scal = meta.tile([128, 9, KCH, 4], F16)

            def scal_fold(hk):
                kr = slice(hk * KH, (hk + 1) * KH)
                for n2 in range(9):
                    mb4 = bass.AP(
                        tensor=mpm[:].tensor,
                        offset=mpm[:].offset + n2 + hk * KH * 9,
                        ap=[list(mpm[:].ap[0]), [9, KH], [0, 4]],
                    )
                    nc.vector.tensor_mul(scal[:, n2, kr], sb4[:, n2, kr], mb4)

            conv1_half(0)
            meta_half(0)
            conv2_half(0)
            scal_fold(0)
            conv1_half(1)
            meta_half(1)
            conv2_half(1)
            scal_fold(1)

            # ---- per spatial-half: gather + combine + transpose; then main conv
            for sp in range(2):
                vc = vbuf.tile([C, 9, 16 * 128], F16, tag="vc")
                paccs = [
                    obuf.tile([128, 16 * 128], F16, tag=f"pacc{hf}", bufs=1,
                              name=f"pacc{hf}")
                    for hf in range(2)
                ]
                for n2 in range(9):
                    g = gbuf.tile([128, 16, 128, 4], F16, tag="g")
                    for kk in range(16):
                        k = sp * 16 + kk
                        dst = bass.AP(
                            tensor=g[:].tensor,
                            offset=g[:].offset + kk * 512,
                            ap=[list(g[:].ap[0]), [1, 512]],
                        )
                        nc.gpsimd.indirect_dma_start(
                            out=dst, out_offset=None,
                            in_=d["ptab"][:],
                            in_offset=bass.IndirectOffsetOnAxis(
                                ap=idxt[:, n2, k:k + 1], axis=0),
                        )
                    # per-piece combine chains (all packed fp16 at 2x except
                    # the strided final add); piece i completes while piece
                    # i+1 gathers. Last set runs at quarter granularity with
                    # the tail main conv emitted per piece.
                    tmp = big.tile([128, 16, 128, 2], F16, tag="big")
                    va = gbuf.tile([128, 16, 128], F16, tag="va")
                    last = (sp == 1 and n2 == 8)
                    np_ = 4 if last else 2
                    kw = 16 // np_
                    if last:
                        outsbs = [
                            obuf.tile([128, 16 * 128], F16, tag=f"osb{hf}",
                                      bufs=1, name=f"outsb{hf}")
                            for hf in range(2)
                        ]
                    for h in range(np_):
                        gm = bass.AP(
                            tensor=g[:].tensor,
                            offset=g[:].offset + h * kw * 512,
                            ap=[list(g[:].ap[0]), [512, kw], [4, 128], [1, 4]],
                        )
                        sc = bass.AP(
                            tensor=scal[:].tensor,
                            offset=scal[:].offset + n2 * (KCH * 4)
                            + (sp * 16 + h * kw) * 4,
                            ap=[list(scal[:].ap[0]), [4, kw], [0, 128], [1, 4]],
                        )
                        nc.vector.tensor_mul(gm, gm, sc)
                        a0 = bass.AP(
                            tensor=g[:].tensor,
                            offset=g[:].offset + h * kw * 512,
                            ap=[list(g[:].ap[0]), [512, kw], [4, 128], [1, 2]],
                        )
                        a1 = bass.AP(
                            tensor=g[:].tensor,
                            offset=g[:].offset + h * kw * 512 + 2,
                            ap=[list(g[:].ap[0]), [512, kw], [4, 128], [1, 2]],
                        )
                        to = bass.AP(
                            tensor=tmp[:].tensor,
                            offset=tmp[:].offset + h * kw * 256,
                            ap=[list(tmp[:].ap[0]), [256, kw], [2, 128], [1, 2]],
                        )
                        nc.vector.tensor_add(to, a0, a1)
                        t0 = bass.AP(
                            tensor=tmp[:].tensor,
                            offset=tmp[:].offset + h * kw * 256,
                            ap=[list(tmp[:].ap[0]), [256, kw], [2, 128]],
                        )
                        t1 = bass.AP(
                            tensor=tmp[:].tensor,
                            offset=tmp[:].offset + h * kw * 256 + 1,
                            ap=[list(tmp[:].ap[0]), [256, kw], [2, 128]],
                        )
                        nc.vector.tensor_add(va[:, h * kw:(h + 1) * kw, :],
                                             t0, t1)
                        for kk in range(h * kw, h * kw + kw):
                            ptv = pst.tile([128, 128], F16, tag="tv")
                            nc.tensor.transpose(ptv[:], va[:, kk, :], id16[:])
                            nc.scalar.copy(vc[:, n2, kk * 128:(kk + 1) * 128],
                                           ptv[:])
                        if last:
                            # tail main conv for the tl(s) this piece completes
                            tls = range(h, h + 1) if np_ == 4 else \
                                range(2 * h, 2 * h + 2)
                            for tl in tls:
                                for hf in range(2):
                                    acc = psp.tile([128, 512], F32, tag="mm")
                                    nc.tensor.matmul(
                                        acc[:], id16[:],
                                        paccs[hf][:, tl * 512:(tl + 1) * 512],
                                        start=True, stop=False)
                                    for j in (7, 8):
                                        nc.tensor.matmul(
                                            acc[:], w2[:, j, hf, :],
                                            vc[:, j, tl * 512:(tl + 1) * 512],
                                            start=False, stop=(j == 8))
                                    nc.scalar.copy(
                                        outsbs[hf][:, tl * 512:(tl + 1) * 512],
                                        acc[:])
                                if tl in (1, 3):
                                    for hf in range(2):
                                        nc.sync.dma_start(
                                            d["out"][128 * hf:128 * (hf + 1),
                                                     2048 * sp + (tl - 1) * 512:
                                                     2048 * sp + (tl + 1) * 512],
                                            outsbs[hf][:, (tl - 1) * 512:
                                                       (tl + 1) * 512])
                    if n2 == 6:
                        # partial main conv over n2=0..6 while last gathers run
                        for hf in range(2):
                            for tl in range(4):
                                acc = psp.tile([128, 512], F32, tag="mm")
                                for j in range(7):
                                    nc.tensor.matmul(
                                        acc[:], w2[:, j, hf, :],
                                        vc[:, j, tl * 512:(tl + 1) * 512],
                                        start=(j == 0), stop=(j == 6))
                                nc.scalar.copy(
                                    paccs[hf][:, tl * 512:(tl + 1) * 512],
                                    acc[:])
                # main conv close-out: sp0 runs the full tail here (hidden
                # under sp1 gathers); sp1 already emitted per-piece above.
                if sp == 0:
                    for hf in range(2):
                        outsb = obuf.tile([128, 16 * 128], F16, tag=f"osb{hf}",
                                          bufs=1, name=f"outsb{hf}")
                        for tl in range(4):
                            acc = psp.tile([128, 512], F32, tag="mm")
                            nc.tensor.matmul(
                                acc[:], id16[:],
                                paccs[hf][:, tl * 512:(tl + 1) * 512],
                                start=True, stop=False)
                            for j in (7, 8):
                                nc.tensor.matmul(
                                    acc[:], w2[:, j, hf, :],
                                    vc[:, j, tl * 512:(tl + 1) * 512],
                                    start=False, stop=(j == 8))
                            nc.scalar.copy(outsb[:, tl * 512:(tl + 1) * 512],
                                           acc[:])
                        nc.sync.dma_start(
                            d["out"][128 * hf:128 * (hf + 1), 0:2048],
                            outsb[:])

    nc.compile()
    _CACHE["nc"] = nc
    return nc


def _host_inputs(b_x, offset_w, offset_b, mod_w, mod_b, conv_w):
    hc = _build_host_constants()
    img = b_x.astype(np.float32)
    imgT = np.ascontiguousarray(img.transpose(0, 2, 1))
    wom = np.zeros((9, C, 18), np.float16)
    wmt = np.zeros((9, C, 9), np.float16)
    for t in range(9):
        dy, dx = t // 3, t % 3
        wom[t] = offset_w[:, :, dy, dx].T
        wmt[3 * dx + dy] = mod_w[:, :, dy, dx].T
    wom = np.ascontiguousarray(wom.transpose(1, 0, 2)).reshape(C, 9 * 18)
    wmt = np.ascontiguousarray(wmt.transpose(1, 0, 2)).reshape(C, 9 * 9)
    w2 = np.zeros((9, 2, C, 128), np.float16)
    for n2 in range(9):
        a2, e2 = n2 // 3, n2 % 3
        for hf in range(2):
            w2[n2, hf] = conv_w[128 * hf:128 * (hf + 1), :, a2, e2].T.astype(
                np.float16)
    w2 = np.ascontiguousarray(w2.transpose(2, 0, 1, 3)).reshape(C, 9 * 2 * 128)
    return {
        "xpad": _pad66(img),
        "xtpad": _pad66(imgT),
        "ptab": _patch_table(img),
        "wom": wom,
        "wmt": wmt,
        "ob": offset_b.reshape(18, 1).astype(np.float32),
        "mb": mod_b.reshape(9, 1).astype(np.float32),
        "sel": hc["sel"],
        "basey": hc["basey"],
        "basex": hc["basex"],
        "w2": w2,
        "id16": hc["ident16"],
    }


def kernel(x, offset_w, offset_b, mod_w, mod_b, conv_w):
    nc = _build_program()
    in_maps = [
        _host_inputs(x[b], offset_w, offset_b, mod_w, mod_b, conv_w)
        for b in range(B)
    ]
    res = run_bass_kernel_spmd(nc, in_maps, core_ids=list(range(B)))
    out = np.stack([
        res.results[b]["out"].reshape(OUT, W, H).transpose(0, 2, 1)
        for b in range(B)
    ])
    return out.astype(np.float32)


if __name__ == "__main__":
    rng = np.random.default_rng(0)
    ins = {
        "x": rng.standard_normal((B, C, H, W), dtype=np.float32),
        "offset_w": (rng.standard_normal((18, C, 3, 3)) / 34).astype(np.float32),
        "offset_b": (rng.standard_normal(18) * 0.01).astype(np.float32),
        "mod_w": (rng.standard_normal((9, C, 3, 3)) / 34).astype(np.float32),
        "mod_b": (rng.standard_normal(9) * 0.01).astype(np.float32),
        "conv_w": (rng.standard_normal((OUT, C, 3, 3)) / 34).astype(np.float32),
    }
    o = kernel(**ins)
    print("out", o.shape, o.dtype, np.abs(o).max())


# revision 35
# speedup vs baseline: 1.0187x; 1.0187x over previous
# Deformable-conv (DCNv2-style, scrambled-reshape variant) Trainium2 Bass kernel.
# Data-parallel over batch: 8 samples -> 8 NeuronCores.
#
# Per-core pipeline (all layouts derived + validated against the reference in numpy):
#   1. offset conv (18ch, fp16) over padded x -> PE-transpose -> per-n2 selection
#      matmuls -> flat 2x2-patch index f00 + bilinear fracs; gathers can start as
#      soon as idxt[n2] lands. Modulation conv (9ch) over padded x^T + scale
#      table build run behind the first gathers.
#   2. 16 indirect-DMA gathers per (sp, n2) from a host-built patch table
#      (row f = [128 ch x 4 corners] of flat pixels [f, f+1, f+64, f+65], fp16,
#      corner-minor so the scale multiply runs at 2x DVE rate).
#   3. Combine: 2 half-tile muls by (modulation x bilinear) scales, 2 half-tile
#      corner-pair adds (all packed fp16, 2x DVE), one strided final add.
#   4. PE-transpose back to channel-major, Act drains PSUM into vc.
#   5. Main conv = 9 accumulated fp16 matmuls per output tile; Act PSUM copies
#      write through a transposed AP to undo the pi2' ordering.
import sys

import numpy as np

sys.path.insert(0, "/opt/trn_rl_repo")

import concourse.bass as bass
import concourse.bacc as bacc
import concourse.mybir as mybir
from concourse import tile
from concourse.bass_utils import run_bass_kernel_spmd

F32 = mybir.dt.float32
F16 = mybir.dt.float16
I32 = mybir.dt.int32

B, C, H, W = 8, 128, 64, 64
OUT = 256
PIX = H * W            # 4096
KCH = 32               # pixel-major chunks (4096 / 128)
TROWS = 4224           # patch table rows (4096 + pad for f+65 reads)

_CACHE = {}


def _build_host_constants():
    if "sel" in _CACHE:
        return _CACHE
    p2 = np.arange(128)
    k2 = np.arange(KCH)
    sel = np.zeros((9, 3, 128, 128), np.float16)   # [n2, r, p_src, p2]
    basey = np.zeros((9, 128, KCH), np.float32)
    basex = np.zeros((9, 128, KCH), np.float32)
    for n2 in range(9):
        a2, e2 = n2 // 3, n2 % 3
        i2 = p2 % 64
        r = (i2 + e2) % 3
        n = 3 * r + a2                       # source kernel point per partition
        J = (64 * e2 + i2) // 3              # source col j per partition
        c_src = 64 * (p2 // 64) + J          # source partition in pixel-major
        for rr in range(3):
            m = r == rr
            sel[n2, rr, c_src[m], p2[m]] = 1.0
        a = n // 3
        e = n % 3
        # y_u = i + a + o_y ; i = j2 = 2*k2 + p2//64
        basey[n2] = (2 * k2[None, :] + (p2 // 64)[:, None]) + a[:, None]
        basex[n2] = (J + e)[:, None] * np.ones((1, KCH), np.float32)
    _CACHE["sel"] = np.ascontiguousarray(
        sel.transpose(2, 0, 1, 3)).reshape(128, 9 * 3 * 128)
    _CACHE["basey"] = np.ascontiguousarray(
        basey.transpose(1, 0, 2)).reshape(128, 9 * KCH)
    _CACHE["basex"] = np.ascontiguousarray(
        basex.transpose(1, 0, 2)).reshape(128, 9 * KCH)
    _CACHE["ident16"] = np.eye(128, dtype=np.float16)
    return _CACHE


def _pad66(img):  # [C,64,64] -> [C, 66*66] zero-padded
    p = np.zeros((C, 66, 66), np.float16)
    p[:, 1:65, 1:65] = img
    return p.reshape(C, 66 * 66)


def _patch_table(img):  # [C,64,64] f32 -> [TROWS, 512] fp16, rows [ch, corner]
    flat = np.zeros((C, TROWS + 65), np.float16)
    flat[:, :PIX] = img.reshape(C, PIX).astype(np.float16)
    f = np.arange(TROWS)
    tab = np.stack(
        [flat[:, f], flat[:, f + 1], flat[:, f + 64], flat[:, f + 65]], axis=2
    )  # [C, TROWS, 4]
    return np.ascontiguousarray(tab.transpose(1, 0, 2)).reshape(TROWS, 512)


def _build_program():
    if "nc" in _CACHE:
        return _CACHE["nc"]
    nc = bacc.Bacc()
    d = {}
    d["xpad"] = nc.dram_tensor("xpad", [C, 66 * 66], F16, kind="ExternalInput")
    d["xtpad"] = nc.dram_tensor("xtpad", [C, 66 * 66], F16, kind="ExternalInput")
    d["ptab"] = nc.dram_tensor("ptab", [TROWS, 512], F16, kind="ExternalInput")
    d["wom"] = nc.dram_tensor("wom", [C, 9 * 18], F16, kind="ExternalInput")
    d["wmt"] = nc.dram_tensor("wmt", [C, 9 * 9], F16, kind="ExternalInput")
    d["ob"] = nc.dram_tensor("ob", [18, 1], F32, kind="ExternalInput")
    d["mb"] = nc.dram_tensor("mb", [9, 1], F32, kind="ExternalInput")
    d["sel"] = nc.dram_tensor("sel", [128, 9 * 3 * 128], F16, kind="ExternalInput")
    d["basey"] = nc.dram_tensor("basey", [128, 9 * KCH], F32, kind="ExternalInput")
    d["basex"] = nc.dram_tensor("basex", [128, 9 * KCH], F32, kind="ExternalInput")
    d["w2"] = nc.dram_tensor("w2", [C, 9 * 2 * 128], F16, kind="ExternalInput")
    d["id16"] = nc.dram_tensor("id16", [128, 128], F16, kind="ExternalInput")
    d["out"] = nc.dram_tensor("out", [OUT, PIX], F16, kind="ExternalOutput")

    AO = mybir.AluOpType

    with tile.TileContext(nc) as tc:
        with (
            tc.tile_pool(name="imgs", bufs=1) as imgs,
            tc.tile_pool(name="wts", bufs=1) as wts,
            tc.tile_pool(name="meta", bufs=1) as meta,
            tc.tile_pool(name="big", bufs=2) as big,
            tc.tile_pool(name="ps", bufs=2, space="PSUM") as psp,
            tc.tile_pool(name="pst", bufs=2, space="PSUM") as pst,
            tc.tile_pool(name="gbuf", bufs=2) as gbuf,
            tc.tile_pool(name="vbuf", bufs=2) as vbuf,
            tc.tile_pool(name="obuf", bufs=2) as obuf,
        ):
            # ---- loads: conv1/idx path first, conv2/scale + main-conv later
            xpad = imgs.tile([C, 66 * 66], F16)
            xtpad = imgs.tile([C, 66 * 66], F16)
            wom = wts.tile([C, 9, 18], F16)
            wmt = wts.tile([C, 9, 9], F16)
            ob = wts.tile([18, 1], F32)
            mb = wts.tile([9, 1], F32)
            id16 = wts.tile([128, 128], F16)
            selt = wts.tile([128, 9, 3, 128], F16)
            basey = wts.tile([128, 9, KCH], F32)
            basex = wts.tile([128, 9, KCH], F32)
            w2 = wts.tile([C, 9, 2, 128], F16)
            nc.sync.dma_start(id16[:], d["id16"][:])
            nc.sync.dma_start(xpad[:, 0:2244], d["xpad"][:, 0:2244])
            nc.sync.dma_start(wom[:], d["wom"][:])
            nc.sync.dma_start(ob[:], d["ob"][:])
            nc.sync.dma_start(xpad[:, 2244:], d["xpad"][:, 2244:])
            nc.sync.dma_start(selt[:], d["sel"][:])
            nc.sync.dma_start(basey[:], d["basey"][:])
            nc.sync.dma_start(basex[:], d["basex"][:])
            nc.sync.dma_start(wmt[:], d["wmt"][:])
            nc.sync.dma_start(mb[:], d["mb"][:])
            nc.sync.dma_start(xtpad[:], d["xtpad"][:])
            nc.sync.dma_start(w2[:], d["w2"][:])

            # ---- PE p-state warmup on the identity while xpad streams in
            for _ in range(24):
                wpt = pst.tile([128, 128], F16, tag="tv", name="wpt")
                nc.tensor.transpose(wpt[:], id16[:], id16[:])

            # ---- conv1 (offsets, 18ch over xpad), transposes interleaved
            ocm = big.tile([128, PIX], F16, tag="big")
            opm = meta.tile([128, KCH, 18], F16)   # pi = 128k+p

            def conv1_range(tl0, tl1):
                for tl in range(tl0, tl1):
                    po = psp.tile([18, 512], F32, tag="mm", name="po")
                    for t in range(9):
                        dy, dx = t // 3, t % 3
                        off = dy * 66 + dx + tl * 8 * 66
                        rhs1 = bass.AP(
                            tensor=xpad[:].tensor, offset=xpad[:].offset + off,
                            ap=[list(xpad[:].ap[0]), [66, 8], [1, 64]],
                        )
                        nc.tensor.matmul(po[:], wom[:, t, :], rhs1,
                                         start=(t == 0), stop=(t == 8))
                    nc.scalar.activation(ocm[0:18, tl * 512:(tl + 1) * 512],
                                         po[:],
                                         mybir.ActivationFunctionType.Identity,
                                         bias=ob[:], scale=1.0)
                    for k in range(4 * tl, 4 * tl + 4):
                        pt = pst.tile([128, 128], F16, tag="tr", name="pt")
                        nc.tensor.transpose(pt[:], ocm[:, k * 128:(k + 1) * 128],
                                            id16[:])
                        nc.scalar.copy(opm[:, k, :], pt[:, 0:18])

            # ---- per-n2: selection matmuls -> positions -> idx + fracs.
            # Split by k-half: half 0 covers sp=0's chunks, so its gathers
            # start after only half the pipeline latency.
            idxt = meta.tile([128, 9, KCH], I32)
            sb4 = meta.tile([128, 9, KCH, 4], F32)
            KH = KCH // 2

            KQ = 8

            def meta_range(kq):
                for n2 in range(9):
                    oyx = pst.tile([128, KQ, 2], F32, tag="oyx", name="oyx")
                    for r in range(3):
                        a2 = n2 // 3
                        ch = 3 * r + a2
                        rhs = bass.AP(
                            tensor=opm[:].tensor,
                            offset=opm[:].offset + ch + kq * KQ * 18,
                            ap=[list(opm[:].ap[0]), [18, KQ], [9, 2]],
                        )
                        nc.tensor.matmul(oyx[:], selt[:, n2, r, :], rhs,
                                         start=(r == 0), stop=(r == 2))
                    kr = slice(kq * KQ, (kq + 1) * KQ)
                    P = meta.tile([128, KQ, 2], F32, tag="P", name="P")
                    nc.vector.tensor_add(P[:, :, 0], oyx[:, :, 0],
                                         basey[:, n2, kr])
                    nc.vector.tensor_add(P[:, :, 1], oyx[:, :, 1],
                                         basex[:, n2, kr])
                    nc.vector.tensor_scalar(P[:], P[:], 0.0, 63.0,
                                            AO.max, AO.min)
                    R0 = meta.tile([128, KQ, 2], F32, tag="R0", name="R0")
                    nc.vector.tensor_scalar(R0[:], P[:], -0.5, 12582912.0,
                                            AO.add, AO.add)
                    nc.vector.tensor_scalar_add(R0[:], R0[:], -12582912.0)
                    Fh = meta.tile([128, KQ, 2], F32, tag="Fh", name="Fh")
                    nc.vector.tensor_sub(Fh[:], P[:], R0[:])
                    nc.vector.scalar_tensor_tensor(
                        idxt[:, n2, kr], R0[:, :, 1], 64.0, R0[:, :, 0],
                        AO.mult, AO.add)
                    # bilinear-only corner weights (modulation folded later):
                    # c0=(1-F1)(1-F0) c1=(1-F1)F0 c2=F1(1-F0) c3=F1*F0
                    nc.vector.tensor_scalar(sb4[:, n2, kr, 0], Fh[:, :, 1],
                                            -1.0, 1.0, AO.mult, AO.add)
                    nc.vector.tensor_mul(sb4[:, n2, kr, 1], sb4[:, n2, kr, 0],
                                         Fh[:, :, 0])
                    nc.vector.tensor_sub(sb4[:, n2, kr, 0], sb4[:, n2, kr, 0],
                                         sb4[:, n2, kr, 1])
                    nc.vector.tensor_mul(sb4[:, n2, kr, 3], Fh[:, :, 1],
                                         Fh[:, :, 0])
                    nc.vector.tensor_sub(sb4[:, n2, kr, 2], Fh[:, :, 1],
                                         sb4[:, n2, kr, 3])

            # ---- conv2 (modulation, 9ch over xtpad) + sigmoid, per-half
            mcm = big.tile([128, PIX], F16, tag="big")
            mpm = meta.tile([128, KCH, 9], F32)    # pi2' = 128k+p

            def conv2_half(hk):
                for tl in range(4 * hk, 4 * hk + 4):
                    pm = psp.tile([9, 512], F32, tag="mm", name="pm")
                    for t in range(9):
                        dy, dx = t // 3, t % 3
                        off = dy * 66 + dx + tl * 8 * 66
                        rhs2 = bass.AP(
                            tensor=xtpad[:].tensor,
                            offset=xtpad[:].offset + off,
                            ap=[list(xtpad[:].ap[0]), [66, 8], [1, 64]],
                        )
                        nc.tensor.matmul(pm[:], wmt[:, t, :], rhs2,
                                         start=(t == 0), stop=(t == 8))
                    nc.scalar.activation(mcm[0:9, tl * 512:(tl + 1) * 512],
                                         pm[:],
                                         mybir.ActivationFunctionType.Sigmoid,
                                         bias=mb[:], scale=1.0)
                    for k in range(4 * tl, 4 * tl + 4):
                        pt2 = pst.tile([128, 128], F16, tag="tr", name="pt2")
                        nc.tensor.transpose(pt2[:],
                                            mcm[:, k * 128:(k + 1) * 128],
                                            id16[:])
                        nc.scalar.copy(mpm[:, k, :], pt2[:, 0:9])

            # ---- scales: fold modulation into bilinear weights (tiny
            # broadcast muls; conv2-dependent work kept off the combine path)
            scal = meta.tile([128, 9, KCH, 4], F16)

            def scal_fold(hk):
                kr = slice(hk * KH, (hk + 1) * KH)
                for n2 in range(9):
                    mb4 = bass.AP(
                        tensor=mpm[:].tensor,
                        offset=mpm[:].offset + n2 + hk * KH * 9,
                        ap=[list(mpm[:].ap[0]), [9, KH], [0, 4]],
                    )
                    nc.vector.tensor_mul(scal[:, n2, kr], sb4[:, n2, kr], mb4)

            conv1_range(0, 2)
            meta_range(0)
            conv1_range(2, 4)
            meta_range(1)
            conv2_half(0)
            scal_fold(0)
            conv1_range(4, 8)
            meta_range(2)
            meta_range(3)
            conv2_half(1)
            scal_fold(1)

            # ---- per spatial-half: gather + combine + transpose; then main conv
            for sp in range(2):
                vc = vbuf.tile([C, 9, 16 * 128], F16, tag="vc")
                paccs = [
                    obuf.tile([128, 16 * 128], F16, tag=f"pacc{hf}", bufs=1,
                              name=f"pacc{hf}")
                    for hf in range(2)
                ]
                for n2 in range(9):
                    g = gbuf.tile([128, 16, 128, 4], F16, tag="g")
                    for kk in range(16):
                        k = sp * 16 + kk
                        dst = bass.AP(
                            tensor=g[:].tensor,
                            offset=g[:].offset + kk * 512,
                            ap=[list(g[:].ap[0]), [1, 512]],
                        )
                        nc.gpsimd.indirect_dma_start(
                            out=dst, out_offset=None,
                            in_=d["ptab"][:],
                            in_offset=bass.IndirectOffsetOnAxis(
                                ap=idxt[:, n2, k:k + 1], axis=0),
                        )
                    # per-piece combine chains (all packed fp16 at 2x except
                    # the strided final add); piece i completes while piece
                    # i+1 gathers. Last set runs at quarter granularity with
                    # the tail main conv emitted per piece.
                    tmp = big.tile([128, 16, 128, 2], F16, tag="big")
                    va = gbuf.tile([128, 16, 128], F16, tag="va")
                    last = (sp == 1 and n2 == 8)
                    np_ = 4 if last else 2
                    kw = 16 // np_
                    if last:
                        outsbs = [
                            obuf.tile([128, 16 * 128], F16, tag=f"osb{hf}",
                                      bufs=1, name=f"outsb{hf}")
                            for hf in range(2)
                        ]
                    for h in range(np_):
                        gm = bass.AP(
                            tensor=g[:].tensor,
                            offset=g[:].offset + h * kw * 512,
                            ap=[list(g[:].ap[0]), [512, kw], [4, 128], [1, 4]],
                        )
                        sc = bass.AP(
                            tensor=scal[:].tensor,
                            offset=scal[:].offset + n2 * (KCH * 4)
                            + (sp * 16 + h * kw) * 4,
                            ap=[list(scal[:].ap[0]), [4, kw], [0, 128], [1, 4]],
                        )
                        nc.vector.tensor_mul(gm, gm, sc)
                        a0 = bass.AP(
                            tensor=g[:].tensor,
                            offset=g[:].offset + h * kw * 512,
                            ap=[list(g[:].ap[0]), [512, kw], [4, 128], [1, 2]],
                        )
                        a1 = bass.AP(
                            tensor=g[:].tensor,
                            offset=g[:].offset + h * kw * 512 + 2,
                            ap=[list(g[:].ap[0]), [512, kw], [4, 128], [1, 2]],
                        )
                        to = bass.AP(
                            tensor=tmp[:].tensor,
                            offset=tmp[:].offset + h * kw * 256,
                            ap=[list(tmp[:].ap[0]), [256, kw], [2, 128], [1, 2]],
                        )
                        nc.vector.tensor_add(to, a0, a1)
                        t0 = bass.AP(
                            tensor=tmp[:].tensor,
                            offset=tmp[:].offset + h * kw * 256,
                            ap=[list(tmp[:].ap[0]), [256, kw], [2, 128]],
                        )
                        t1 = bass.AP(
                            tensor=tmp[:].tensor,
                            offset=tmp[:].offset + h * kw * 256 + 1,
                            ap=[list(tmp[:].ap[0]), [256, kw], [2, 128]],
                        )
                        nc.vector.tensor_add(va[:, h * kw:(h + 1) * kw, :],
                                             t0, t1)
                        for kk in range(h * kw, h * kw + kw):
                            ptv = pst.tile([128, 128], F16, tag="tv")
                            nc.tensor.transpose(ptv[:], va[:, kk, :], id16[:])
                            nc.scalar.copy(vc[:, n2, kk * 128:(kk + 1) * 128],
                                           ptv[:])
                        if last:
                            # tail main conv for the tl(s) this piece completes
                            tls = range(h, h + 1) if np_ == 4 else \
                                range(2 * h, 2 * h + 2)
                            for tl in tls:
                                for hf in range(2):
                                    acc = psp.tile([128, 512], F32, tag="mm")
                                    nc.tensor.matmul(
                                        acc[:], id16[:],
                                        paccs[hf][:, tl * 512:(tl + 1) * 512],
                                        start=True, stop=False)
                                    for j in (7, 8):
                                        nc.tensor.matmul(
                                            acc[:], w2[:, j, hf, :],
                                            vc[:, j, tl * 512:(tl + 1) * 512],
                                            start=False, stop=(j == 8))
                                    nc.scalar.copy(
                                        outsbs[hf][:, tl * 512:(tl + 1) * 512],
                                        acc[:])
                                if tl in (1, 3):
                                    for hf in range(2):
                                        nc.sync.dma_start(
                                            d["out"][128 * hf:128 * (hf + 1),
                                                     2048 * sp + (tl - 1) * 512:
                                                     2048 * sp + (tl + 1) * 512],
                                            outsbs[hf][:, (tl - 1) * 512:
                                                       (tl + 1) * 512])
                    if n2 == 6:
                        # partial main conv over n2=0..6 while last gathers run
                        for hf in range(2):
                            for tl in range(4):
                                acc = psp.tile([128, 512], F32, tag="mm")
                                for j in range(7):
                                    nc.tensor.matmul(
                                        acc[:], w2[:, j, hf, :],
                                        vc[:, j, tl * 512:(tl + 1) * 512],
                                        start=(j == 0), stop=(j == 6))
                                nc.scalar.copy(
                                    paccs[hf][:, tl * 512:(tl + 1) * 512],
                                    acc[:])
                # main conv close-out: sp0 runs the full tail here (hidden
                # under sp1 gathers); sp1 already emitted per-piece above.
                if sp == 0:
                    for hf in range(2):
                        outsb = obuf.tile([128, 16 * 128], F16, tag=f"osb{hf}",
                                          bufs=1, name=f"outsb{hf}")
                        for tl in range(4):
                            acc = psp.tile([128, 512], F32, tag="mm")
                            nc.tensor.matmul(
                                acc[:], id16[:],
                                paccs[hf][:, tl * 512:(tl + 1) * 512],
                                start=True, stop=False)
                            for j in (7, 8):
                                nc.tensor.matmul(
                                    acc[:], w2[:, j, hf, :],
                                    vc[:, j, tl * 512:(tl + 1) * 512],
                                    start=False, stop=(j == 8))
                            nc.scalar.copy(outsb[:, tl * 512:(tl + 1) * 512],
                                           acc[:])
                        nc.sync.dma_start(
                            d["out"][128 * hf:128 * (hf + 1), 0:2048],
                            outsb[:])

    nc.compile()
    _CACHE["nc"] = nc
    return nc


def _host_inputs(b_x, offset_w, offset_b, mod_w, mod_b, conv_w):
    hc = _build_host_constants()
    img = b_x.astype(np.float32)
    imgT = np.ascontiguousarray(img.transpose(0, 2, 1))
    wom = np.zeros((9, C, 18), np.float16)
    wmt = np.zeros((9, C, 9), np.float16)
    for t in range(9):
        dy, dx = t // 3, t % 3
        wom[t] = offset_w[:, :, dy, dx].T
        wmt[3 * dx + dy] = mod_w[:, :, dy, dx].T
    wom = np.ascontiguousarray(wom.transpose(1, 0, 2)).reshape(C, 9 * 18)
    wmt = np.ascontiguousarray(wmt.transpose(1, 0, 2)).reshape(C, 9 * 9)
    w2 = np.zeros((9, 2, C, 128), np.float16)
    for n2 in range(9):
        a2, e2 = n2 // 3, n2 % 3
        for hf in range(2):
            w2[n2, hf] = conv_w[128 * hf:128 * (hf + 1), :, a2, e2].T.astype(
                np.float16)
    w2 = np.ascontiguousarray(w2.transpose(2, 0, 1, 3)).reshape(C, 9 * 2 * 128)
    return {
        "xpad": _pad66(img),
        "xtpad": _pad66(imgT),
        "ptab": _patch_table(img),
        "wom": wom,
        "wmt": wmt,
        "ob": offset_b.reshape(18, 1).astype(np.float32),
        "mb": mod_b.reshape(9, 1).astype(np.float32),
        "sel": hc["sel"],
        "basey": hc["basey"],
        "basex": hc["basex"],
        "w2": w2,
        "id16": hc["ident16"],
    }


def kernel(x, offset_w, offset_b, mod_w, mod_b, conv_w):
    nc = _build_program()
    in_maps = [
        _host_inputs(x[b], offset_w, offset_b, mod_w, mod_b, conv_w)
        for b in range(B)
    ]
    res = run_bass_kernel_spmd(nc, in_maps, core_ids=list(range(B)))
    out = np.stack([
        res.results[b]["out"].reshape(OUT, W, H).transpose(0, 2, 1)
        for b in range(B)
    ])
    return out.astype(np.float32)


if __name__ == "__main__":
    rng = np.random.default_rng(0)
    ins = {
        "x": rng.standard_normal((B, C, H, W), dtype=np.float32),
        "offset_w": (rng.standard_normal((18, C, 3, 3)) / 34).astype(np.float32),
        "offset_b": (rng.standard_normal(18) * 0.01).astype(np.float32),
        "mod_w": (rng.standard_normal((9, C, 3, 3)) / 34).astype(np.float32),
        "mod_b": (rng.standard_normal(9) * 0.01).astype(np.float32),
        "conv_w": (rng.standard_normal((OUT, C, 3, 3)) / 34).astype(np.float32),
    }
    o = kernel(**ins)
    print("out", o.shape, o.dtype, np.abs(o).max())


# revision 36
# speedup vs baseline: 1.0220x; 1.0032x over previous
# Deformable-conv (DCNv2-style, scrambled-reshape variant) Trainium2 Bass kernel.
# Data-parallel over batch: 8 samples -> 8 NeuronCores.
#
# Per-core pipeline (all layouts derived + validated against the reference in numpy):
#   1. offset conv (18ch, fp16) over padded x -> PE-transpose -> per-n2 selection
#      matmuls -> flat 2x2-patch index f00 + bilinear fracs; gathers can start as
#      soon as idxt[n2] lands. Modulation conv (9ch) over padded x^T + scale
#      table build run behind the first gathers.
#   2. 16 indirect-DMA gathers per (sp, n2) from a host-built patch table
#      (row f = [128 ch x 4 corners] of flat pixels [f, f+1, f+64, f+65], fp16,
#      corner-minor so the scale multiply runs at 2x DVE rate).
#   3. Combine: 2 half-tile muls by (modulation x bilinear) scales, 2 half-tile
#      corner-pair adds (all packed fp16, 2x DVE), one strided final add.
#   4. PE-transpose back to channel-major, Act drains PSUM into vc.
#   5. Main conv = 9 accumulated fp16 matmuls per output tile; Act PSUM copies
#      write through a transposed AP to undo the pi2' ordering.
import sys

import numpy as np

sys.path.insert(0, "/opt/trn_rl_repo")

import concourse.bass as bass
import concourse.bacc as bacc
import concourse.mybir as mybir
from concourse import tile
from concourse.bass_utils import run_bass_kernel_spmd

F32 = mybir.dt.float32
F16 = mybir.dt.float16
I32 = mybir.dt.int32

B, C, H, W = 8, 128, 64, 64
OUT = 256
PIX = H * W            # 4096
KCH = 32               # pixel-major chunks (4096 / 128)
TROWS = 4224           # patch table rows (4096 + pad for f+65 reads)

_CACHE = {}


def _build_host_constants():
    if "sel" in _CACHE:
        return _CACHE
    p2 = np.arange(128)
    k2 = np.arange(KCH)
    sel = np.zeros((9, 3, 128, 128), np.float16)   # [n2, r, p_src, p2]
    basey = np.zeros((9, 128, KCH), np.float32)
    basex = np.zeros((9, 128, KCH), np.float32)
    for n2 in range(9):
        a2, e2 = n2 // 3, n2 % 3
        i2 = p2 % 64
        r = (i2 + e2) % 3
        n = 3 * r + a2                       # source kernel point per partition
        J = (64 * e2 + i2) // 3              # source col j per partition
        c_src = 64 * (p2 // 64) + J          # source partition in pixel-major
        for rr in range(3):
            m = r == rr
            sel[n2, rr, c_src[m], p2[m]] = 1.0
        a = n // 3
        e = n % 3
        # y_u = i + a + o_y ; i = j2 = 2*k2 + p2//64
        basey[n2] = (2 * k2[None, :] + (p2 // 64)[:, None]) + a[:, None]
        basex[n2] = (J + e)[:, None] * np.ones((1, KCH), np.float32)
    _CACHE["sel"] = np.ascontiguousarray(
        sel.transpose(2, 0, 1, 3)).reshape(128, 9 * 3 * 128)
    _CACHE["basey"] = np.ascontiguousarray(
        basey.transpose(1, 0, 2)).reshape(128, 9 * KCH)
    _CACHE["basex"] = np.ascontiguousarray(
        basex.transpose(1, 0, 2)).reshape(128, 9 * KCH)
    _CACHE["ident16"] = np.eye(128, dtype=np.float16)
    return _CACHE


def _pad66(img):  # [C,64,64] -> [C, 66*66] zero-padded
    p = np.zeros((C, 66, 66), np.float16)
    p[:, 1:65, 1:65] = img
    return p.reshape(C, 66 * 66)


def _patch_table(img):  # [C,64,64] f32 -> [TROWS, 512] fp16, rows [ch, corner]
    flat = np.zeros((C, TROWS + 65), np.float16)
    flat[:, :PIX] = img.reshape(C, PIX).astype(np.float16)
    f = np.arange(TROWS)
    tab = np.stack(
        [flat[:, f], flat[:, f + 1], flat[:, f + 64], flat[:, f + 65]], axis=2
    )  # [C, TROWS, 4]
    return np.ascontiguousarray(tab.transpose(1, 0, 2)).reshape(TROWS, 512)


def _build_program():
    if "nc" in _CACHE:
        return _CACHE["nc"]
    nc = bacc.Bacc()
    d = {}
    d["xpad"] = nc.dram_tensor("xpad", [C, 66 * 66], F16, kind="ExternalInput")
    d["xtpad"] = nc.dram_tensor("xtpad", [C, 66 * 66], F16, kind="ExternalInput")
    d["ptab"] = nc.dram_tensor("ptab", [TROWS, 512], F16, kind="ExternalInput")
    d["wom"] = nc.dram_tensor("wom", [C, 9 * 18], F16, kind="ExternalInput")
    d["wmt"] = nc.dram_tensor("wmt", [C, 9 * 9], F16, kind="ExternalInput")
    d["ob"] = nc.dram_tensor("ob", [18, 1], F32, kind="ExternalInput")
    d["mb"] = nc.dram_tensor("mb", [9, 1], F32, kind="ExternalInput")
    d["sel"] = nc.dram_tensor("sel", [128, 9 * 3 * 128], F16, kind="ExternalInput")
    d["basey"] = nc.dram_tensor("basey", [128, 9 * KCH], F32, kind="ExternalInput")
    d["basex"] = nc.dram_tensor("basex", [128, 9 * KCH], F32, kind="ExternalInput")
    d["w2"] = nc.dram_tensor("w2", [C, 9 * 2 * 128], F16, kind="ExternalInput")
    d["id16"] = nc.dram_tensor("id16", [128, 128], F16, kind="ExternalInput")
    d["out"] = nc.dram_tensor("out", [OUT, PIX], F16, kind="ExternalOutput")

    AO = mybir.AluOpType

    with tile.TileContext(nc) as tc:
        with (
            tc.tile_pool(name="imgs", bufs=1) as imgs,
            tc.tile_pool(name="wts", bufs=1) as wts,
            tc.tile_pool(name="meta", bufs=1) as meta,
            tc.tile_pool(name="big", bufs=2) as big,
            tc.tile_pool(name="ps", bufs=2, space="PSUM") as psp,
            tc.tile_pool(name="pst", bufs=2, space="PSUM") as pst,
            tc.tile_pool(name="gbuf", bufs=2) as gbuf,
            tc.tile_pool(name="vbuf", bufs=2) as vbuf,
            tc.tile_pool(name="obuf", bufs=2) as obuf,
        ):
            # ---- loads: conv1/idx path first, conv2/scale + main-conv later
            xpad = imgs.tile([C, 66 * 66], F16)
            xtpad = imgs.tile([C, 66 * 66], F16)
            wom = wts.tile([C, 9, 18], F16)
            wmt = wts.tile([C, 9, 9], F16)
            ob = wts.tile([18, 1], F32)
            mb = wts.tile([9, 1], F32)
            id16 = wts.tile([128, 128], F16)
            selt = wts.tile([128, 9, 3, 128], F16)
            basey = wts.tile([128, 9, KCH], F32)
            basex = wts.tile([128, 9, KCH], F32)
            w2 = wts.tile([C, 9, 2, 128], F16)
            nc.sync.dma_start(id16[:], d["id16"][:])
            nc.sync.dma_start(xpad[:, 0:1188], d["xpad"][:, 0:1188])
            nc.sync.dma_start(wom[:], d["wom"][:])
            nc.sync.dma_start(ob[:], d["ob"][:])
            nc.sync.dma_start(xpad[:, 1188:2244], d["xpad"][:, 1188:2244])
            nc.sync.dma_start(xpad[:, 2244:], d["xpad"][:, 2244:])
            nc.sync.dma_start(selt[:], d["sel"][:])
            nc.sync.dma_start(basey[:], d["basey"][:])
            nc.sync.dma_start(basex[:], d["basex"][:])
            nc.sync.dma_start(wmt[:], d["wmt"][:])
            nc.sync.dma_start(mb[:], d["mb"][:])
            nc.sync.dma_start(xtpad[:], d["xtpad"][:])
            nc.sync.dma_start(w2[:], d["w2"][:])

            # ---- PE p-state warmup on the identity while xpad streams in
            for _ in range(24):
                wpt = pst.tile([128, 128], F16, tag="tv", name="wpt")
                nc.tensor.transpose(wpt[:], id16[:], id16[:])

            # ---- conv1 (offsets, 18ch over xpad), transposes interleaved
            ocm = big.tile([128, PIX], F16, tag="big")
            opm = meta.tile([128, KCH, 18], F16)   # pi = 128k+p

            def conv1_range(tl0, tl1):
                for tl in range(tl0, tl1):
                    po = psp.tile([18, 512], F32, tag="mm", name="po")
                    for t in range(9):
                        dy, dx = t // 3, t % 3
                        off = dy * 66 + dx + tl * 8 * 66
                        rhs1 = bass.AP(
                            tensor=xpad[:].tensor, offset=xpad[:].offset + off,
                            ap=[list(xpad[:].ap[0]), [66, 8], [1, 64]],
                        )
                        nc.tensor.matmul(po[:], wom[:, t, :], rhs1,
                                         start=(t == 0), stop=(t == 8))
                    nc.scalar.activation(ocm[0:18, tl * 512:(tl + 1) * 512],
                                         po[:],
                                         mybir.ActivationFunctionType.Identity,
                                         bias=ob[:], scale=1.0)
                    for k in range(4 * tl, 4 * tl + 4):
                        pt = pst.tile([128, 128], F16, tag="tr", name="pt")
                        nc.tensor.transpose(pt[:], ocm[:, k * 128:(k + 1) * 128],
                                            id16[:])
                        nc.scalar.copy(opm[:, k, :], pt[:, 0:18])

            # ---- per-n2: selection matmuls -> positions -> idx + fracs.
            # Split by k-half: half 0 covers sp=0's chunks, so its gathers
            # start after only half the pipeline latency.
            idxt = meta.tile([128, 9, KCH], I32)
            sb4 = meta.tile([128, 9, KCH, 4], F32)
            KH = KCH // 2

            KQ = 8

            def meta_range(kq):
                for n2 in range(9):
                    oyx = pst.tile([128, KQ, 2], F32, tag="oyx", name="oyx")
                    for r in range(3):
                        a2 = n2 // 3
                        ch = 3 * r + a2
                        rhs = bass.AP(
                            tensor=opm[:].tensor,
                            offset=opm[:].offset + ch + kq * KQ * 18,
                            ap=[list(opm[:].ap[0]), [18, KQ], [9, 2]],
                        )
                        nc.tensor.matmul(oyx[:], selt[:, n2, r, :], rhs,
                                         start=(r == 0), stop=(r == 2))
                    kr = slice(kq * KQ, (kq + 1) * KQ)
                    P = meta.tile([128, KQ, 2], F32, tag="P", name="P")
                    nc.vector.tensor_add(P[:, :, 0], oyx[:, :, 0],
                                         basey[:, n2, kr])
                    nc.vector.tensor_add(P[:, :, 1], oyx[:, :, 1],
                                         basex[:, n2, kr])
                    nc.vector.tensor_scalar(P[:], P[:], 0.0, 63.0,
                                            AO.max, AO.min)
                    R0 = meta.tile([128, KQ, 2], F32, tag="R0", name="R0")
                    nc.vector.tensor_scalar(R0[:], P[:], -0.5, 12582912.0,
                                            AO.add, AO.add)
                    nc.vector.tensor_scalar_add(R0[:], R0[:], -12582912.0)
                    Fh = meta.tile([128, KQ, 2], F32, tag="Fh", name="Fh")
                    nc.vector.tensor_sub(Fh[:], P[:], R0[:])
                    nc.vector.scalar_tensor_tensor(
                        idxt[:, n2, kr], R0[:, :, 1], 64.0, R0[:, :, 0],
                        AO.mult, AO.add)
                    # bilinear-only corner weights (modulation folded later):
                    # c0=(1-F1)(1-F0) c1=(1-F1)F0 c2=F1(1-F0) c3=F1*F0
                    nc.vector.tensor_scalar(sb4[:, n2, kr, 0], Fh[:, :, 1],
                                            -1.0, 1.0, AO.mult, AO.add)
                    nc.vector.tensor_mul(sb4[:, n2, kr, 1], sb4[:, n2, kr, 0],
                                         Fh[:, :, 0])
                    nc.vector.tensor_sub(sb4[:, n2, kr, 0], sb4[:, n2, kr, 0],
                                         sb4[:, n2, kr, 1])
                    nc.vector.tensor_mul(sb4[:, n2, kr, 3], Fh[:, :, 1],
                                         Fh[:, :, 0])
                    nc.vector.tensor_sub(sb4[:, n2, kr, 2], Fh[:, :, 1],
                                         sb4[:, n2, kr, 3])

            # ---- conv2 (modulation, 9ch over xtpad) + sigmoid, per-half
            mcm = big.tile([128, PIX], F16, tag="big")
            mpm = meta.tile([128, KCH, 9], F32)    # pi2' = 128k+p

            def conv2_half(hk):
                for tl in range(4 * hk, 4 * hk + 4):
                    pm = psp.tile([9, 512], F32, tag="mm", name="pm")
                    for t in range(9):
                        dy, dx = t // 3, t % 3
                        off = dy * 66 + dx + tl * 8 * 66
                        rhs2 = bass.AP(
                            tensor=xtpad[:].tensor,
                            offset=xtpad[:].offset + off,
                            ap=[list(xtpad[:].ap[0]), [66, 8], [1, 64]],
                        )
                        nc.tensor.matmul(pm[:], wmt[:, t, :], rhs2,
                                         start=(t == 0), stop=(t == 8))
                    nc.scalar.activation(mcm[0:9, tl * 512:(tl + 1) * 512],
                                         pm[:],
                                         mybir.ActivationFunctionType.Sigmoid,
                                         bias=mb[:], scale=1.0)
                    for k in range(4 * tl, 4 * tl + 4):
                        pt2 = pst.tile([128, 128], F16, tag="tr", name="pt2")
                        nc.tensor.transpose(pt2[:],
                                            mcm[:, k * 128:(k + 1) * 128],
                                            id16[:])
                        nc.scalar.copy(mpm[:, k, :], pt2[:, 0:9])

            # ---- scales: fold modulation into bilinear weights (tiny
            # broadcast muls; conv2-dependent work kept off the combine path)
            scal = meta.tile([128, 9, KCH, 4], F16)

            def scal_fold(hk):
                kr = slice(hk * KH, (hk + 1) * KH)
                for n2 in range(9):
                    mb4 = bass.AP(
                        tensor=mpm[:].tensor,
                        offset=mpm[:].offset + n2 + hk * KH * 9,
                        ap=[list(mpm[:].ap[0]), [9, KH], [0, 4]],
                    )
                    nc.vector.tensor_mul(scal[:, n2, kr], sb4[:, n2, kr], mb4)

            conv1_range(0, 2)
            meta_range(0)
            conv1_range(2, 4)
            meta_range(1)
            conv2_half(0)
            scal_fold(0)
            conv1_range(4, 8)
            meta_range(2)
            meta_range(3)
            conv2_half(1)
            scal_fold(1)

            # ---- per spatial-half: gather + combine + transpose; then main conv
            for sp in range(2):
                vc = vbuf.tile([C, 9, 16 * 128], F16, tag="vc")
                paccs = [
                    obuf.tile([128, 16 * 128], F16, tag=f"pacc{hf}", bufs=1,
                              name=f"pacc{hf}")
                    for hf in range(2)
                ]
                for n2 in range(9):
                    g = gbuf.tile([128, 16, 128, 4], F16, tag="g")
                    for kk in range(16):
                        k = sp * 16 + kk
                        dst = bass.AP(
                            tensor=g[:].tensor,
                            offset=g[:].offset + kk * 512,
                            ap=[list(g[:].ap[0]), [1, 512]],
                        )
                        nc.gpsimd.indirect_dma_start(
                            out=dst, out_offset=None,
                            in_=d["ptab"][:],
                            in_offset=bass.IndirectOffsetOnAxis(
                                ap=idxt[:, n2, k:k + 1], axis=0),
                        )
                    # per-piece combine chains (all packed fp16 at 2x except
                    # the strided final add); piece i completes while piece
                    # i+1 gathers. Last set runs at quarter granularity with
                    # the tail main conv emitted per piece.
                    tmp = big.tile([128, 16, 128, 2], F16, tag="big")
                    va = gbuf.tile([128, 16, 128], F16, tag="va")
                    last = (sp == 1 and n2 == 8)
                    np_ = 4 if last else 2
                    kw = 16 // np_
                    if last:
                        outsbs = [
                            obuf.tile([128, 16 * 128], F16, tag=f"osb{hf}",
                                      bufs=1, name=f"outsb{hf}")
                            for hf in range(2)
                        ]
                    for h in range(np_):
                        gm = bass.AP(
                            tensor=g[:].tensor,
                            offset=g[:].offset + h * kw * 512,
                            ap=[list(g[:].ap[0]), [512, kw], [4, 128], [1, 4]],
                        )
                        sc = bass.AP(
                            tensor=scal[:].tensor,
                            offset=scal[:].offset + n2 * (KCH * 4)
                            + (sp * 16 + h * kw) * 4,
                            ap=[list(scal[:].ap[0]), [4, kw], [0, 128], [1, 4]],
                        )
                        nc.vector.tensor_mul(gm, gm, sc)
                        a0 = bass.AP(
                            tensor=g[:].tensor,
                            offset=g[:].offset + h * kw * 512,
                            ap=[list(g[:].ap[0]), [512, kw], [4, 128], [1, 2]],
                        )
                        a1 = bass.AP(
                            tensor=g[:].tensor,
                            offset=g[:].offset + h * kw * 512 + 2,
                            ap=[list(g[:].ap[0]), [512, kw], [4, 128], [1, 2]],
                        )
                        to = bass.AP(
                            tensor=tmp[:].tensor,
                            offset=tmp[:].offset + h * kw * 256,
                            ap=[list(tmp[:].ap[0]), [256, kw], [2, 128], [1, 2]],
                        )
                        nc.vector.tensor_add(to, a0, a1)
                        t0 = bass.AP(
                            tensor=tmp[:].tensor,
                            offset=tmp[:].offset + h * kw * 256,
                            ap=[list(tmp[:].ap[0]), [256, kw], [2, 128]],
                        )
                        t1 = bass.AP(
                            tensor=tmp[:].tensor,
                            offset=tmp[:].offset + h * kw * 256 + 1,
                            ap=[list(tmp[:].ap[0]), [256, kw], [2, 128]],
                        )
                        nc.vector.tensor_add(va[:, h * kw:(h + 1) * kw, :],
                                             t0, t1)
                        for kk in range(h * kw, h * kw + kw):
                            ptv = pst.tile([128, 128], F16, tag="tv")
                            nc.tensor.transpose(ptv[:], va[:, kk, :], id16[:])
                            nc.scalar.copy(vc[:, n2, kk * 128:(kk + 1) * 128],
                                           ptv[:])
                        if last:
                            # tail main conv for the tl(s) this piece completes
                            tls = range(h, h + 1) if np_ == 4 else \
                                range(2 * h, 2 * h + 2)
                            for tl in tls:
                                for hf in range(2):
                                    acc = psp.tile([128, 512], F32, tag="mm")
                                    nc.tensor.matmul(
                                        acc[:], id16[:],
                                        paccs[hf][:, tl * 512:(tl + 1) * 512],
                                        start=True, stop=False)
                                    for j in (7, 8):
                                        nc.tensor.matmul(
                                            acc[:], w2[:, j, hf, :],
                                            vc[:, j, tl * 512:(tl + 1) * 512],
                                            start=False, stop=(j == 8))
                                    nc.scalar.copy(
                                        outsbs[hf][:, tl * 512:(tl + 1) * 512],
                                        acc[:])
                                if tl == 1:
                                    for hf in range(2):
                                        nc.sync.dma_start(
                                            d["out"][128 * hf:128 * (hf + 1),
                                                     2048 * sp:2048 * sp + 1024],
                                            outsbs[hf][:, 0:1024])
                                elif tl >= 2:
                                    for hf in range(2):
                                        nc.sync.dma_start(
                                            d["out"][128 * hf:128 * (hf + 1),
                                                     2048 * sp + tl * 512:
                                                     2048 * sp + (tl + 1) * 512],
                                            outsbs[hf][:, tl * 512:
                                                       (tl + 1) * 512])
                    if n2 == 6:
                        # partial main conv over n2=0..6 while last gathers run
                        for hf in range(2):
                            for tl in range(4):
                                acc = psp.tile([128, 512], F32, tag="mm")
                                for j in range(7):
                                    nc.tensor.matmul(
                                        acc[:], w2[:, j, hf, :],
                                        vc[:, j, tl * 512:(tl + 1) * 512],
                                        start=(j == 0), stop=(j == 6))
                                nc.scalar.copy(
                                    paccs[hf][:, tl * 512:(tl + 1) * 512],
                                    acc[:])
                # main conv close-out: sp0 runs the full tail here (hidden
                # under sp1 gathers); sp1 already emitted per-piece above.
                if sp == 0:
                    for hf in range(2):
                        outsb = obuf.tile([128, 16 * 128], F16, tag=f"osb{hf}",
                                          bufs=1, name=f"outsb{hf}")
                        for tl in range(4):
                            acc = psp.tile([128, 512], F32, tag="mm")
                            nc.tensor.matmul(
                                acc[:], id16[:],
                                paccs[hf][:, tl * 512:(tl + 1) * 512],
                                start=True, stop=False)
                            for j in (7, 8):
                                nc.tensor.matmul(
                                    acc[:], w2[:, j, hf, :],
                                    vc[:, j, tl * 512:(tl + 1) * 512],
                                    start=False, stop=(j == 8))
                            nc.scalar.copy(outsb[:, tl * 512:(tl + 1) * 512],
                                           acc[:])
                        nc.sync.dma_start(
                            d["out"][128 * hf:128 * (hf + 1), 0:2048],
                            outsb[:])

    nc.compile()
    _CACHE["nc"] = nc
    return nc


def _host_inputs(b_x, offset_w, offset_b, mod_w, mod_b, conv_w):
    hc = _build_host_constants()
    img = b_x.astype(np.float32)
    imgT = np.ascontiguousarray(img.transpose(0, 2, 1))
    wom = np.zeros((9, C, 18), np.float16)
    wmt = np.zeros((9, C, 9), np.float16)
    for t in range(9):
        dy, dx = t // 3, t % 3
        wom[t] = offset_w[:, :, dy, dx].T
        wmt[3 * dx + dy] = mod_w[:, :, dy, dx].T
    wom = np.ascontiguousarray(wom.transpose(1, 0, 2)).reshape(C, 9 * 18)
    wmt = np.ascontiguousarray(wmt.transpose(1, 0, 2)).reshape(C, 9 * 9)
    w2 = np.zeros((9, 2, C, 128), np.float16)
    for n2 in range(9):
        a2, e2 = n2 // 3, n2 % 3
        for hf in range(2):
            w2[n2, hf] = conv_w[128 * hf:128 * (hf + 1), :, a2, e2].T.astype(
                np.float16)
    w2 = np.ascontiguousarray(w2.transpose(2, 0, 1, 3)).reshape(C, 9 * 2 * 128)
    return {
        "xpad": _pad66(img),
        "xtpad": _pad66(imgT),
        "ptab": _patch_table(img),
        "wom": wom,
        "wmt": wmt,
        "ob": offset_b.reshape(18, 1).astype(np.float32),
        "mb": mod_b.reshape(9, 1).astype(np.float32),
        "sel": hc["sel"],
        "basey": hc["basey"],
        "basex": hc["basex"],
        "w2": w2,
        "id16": hc["ident16"],
    }


def kernel(x, offset_w, offset_b, mod_w, mod_b, conv_w):
    nc = _build_program()
    in_maps = [
        _host_inputs(x[b], offset_w, offset_b, mod_w, mod_b, conv_w)
        for b in range(B)
    ]
    res = run_bass_kernel_spmd(nc, in_maps, core_ids=list(range(B)))
    out = np.stack([
        res.results[b]["out"].reshape(OUT, W, H).transpose(0, 2, 1)
        for b in range(B)
    ])
    return out.astype(np.float32)


if __name__ == "__main__":
    rng = np.random.default_rng(0)
    ins = {
        "x": rng.standard_normal((B, C, H, W), dtype=np.float32),
        "offset_w": (rng.standard_normal((18, C, 3, 3)) / 34).astype(np.float32),
        "offset_b": (rng.standard_normal(18) * 0.01).astype(np.float32),
        "mod_w": (rng.standard_normal((9, C, 3, 3)) / 34).astype(np.float32),
        "mod_b": (rng.standard_normal(9) * 0.01).astype(np.float32),
        "conv_w": (rng.standard_normal((OUT, C, 3, 3)) / 34).astype(np.float32),
    }
    o = kernel(**ins)
    print("out", o.shape, o.dtype, np.abs(o).max())
